# revision 41
# baseline (speedup 1.0000x reference)
"""Trainium2 Bass kernel for the mca_g2l sparse-attention module.

Sharding: head-parallel over 8 cores (1 head each). All on-device tensors are
feature-major ("^T": [feature, tokens]); attention is computed key-major
(S^T [keys, queries]) so the softmax denominators come from ones-matmuls and
the AV / ave-branch matmuls need no attention transpose.

Cross-core data movement (3 collectives, all SPMD-symmetric):
  AG-x : AllGather of the fp16 x^T C-row shards (rebuilds full x^T everywhere)
  RS   : ReduceScatter (fp16 add) of [attn_avg^T | raw_cls^T | raw_reg^T],
         grouped per owner core — head-sums for masks + ave branch in one shot
  AG-2 : AllGather (fp16) of masked-exp slices, AV outputs, v^T[:, :N1],
         renorm partials
Raw v-v similarities are computed per-head locally in phase B (each core does
its own head for all 2048 keys) and head-summed by the RS, so no exchange of
normalized v is needed at all.
Output linears are column-sharded (256 cols/core); ave-branch output columns are
head-sharded so `support` is the core's own token-major v. Host assembles the
final [512, 3072] features from per-core column slices.

Inputs ship as a single fp16 blob (~4.9 MB/core — per-exec input staging through
the axon relay costs ~0.67 ms/MB above ~5 MB, which dominated the f32 version).
Blob-fed matmuls run fp16 x fp16; device-internal matmuls run float32r.
"""

import numpy as np

import concourse.bacc as bacc
import concourse.mybir as mybir
import concourse.tile as tile
from concourse.masks import make_identity

F32 = mybir.dt.float32
F32R = mybir.dt.float32r
BF16 = mybir.dt.bfloat16
F16 = mybir.dt.float16
AF = mybir.ActivationFunctionType

N_CORES = 8
N1 = 512
N2 = 2048
C = 1024
HD = 128
SCALE = 25.0
KT = N2 // 128          # 16 key tiles of 128
TT = N2 // 512          # 4 token tiles of 512
CC = C // 128           # 8 contraction chunks
MYK = N2 // N_CORES     # 256 keys owned per core after RS / A2A

# AG-2 payloads (per-rank block, x N1 cols, fp16), split in two so the x/v
# half can fly while the masked-exp half is still being computed:
# AG-2a (ready at end of phase B):
#   [0:128)    x_cls^T * 1/(2*D_cls)      (AV output, half-scaled)
#   [128:256)  x_reg^T * 1/(2*D_reg)
#   [256:384)  v_cls^T[:, :N1]            (x_ori part)
#   [384:512)  v_reg^T[:, :N1]
# AG-2b (after the RS-dependent mask phase):
#   [0:256)    mE_sim  = sim_mask * exp(attn_sum/H)   (my 256 keys)
#   [256:512)  mE_obj  = obj_mask * mE_sim
#   [512:514)  D partials: row0 = sum_k mE_sim, row1 = sum_k mE_obj
AG2A_ROWS = 512
AG2B_ROWS = 514
EXP_SHIFT = -4.0                        # exp(logit-4): softmax-invariant shift
                                        # keeping unnormalized exps in fp16 range

# RS payload: per owner core c, rows c*768..(c+1)*768 of rs_in hold that core's
# 256 keys as [attn_avg^T 256 | raw_cls^T 256 | raw_reg^T 256], fp16.
RS_BLK = 768

# packed input blob layout (rows x 512 fp16). x^T is sharded: each core ships
# its 128 C-rows of xt_cls+xt_reg; an on-device AllGather rebuilds the full x^T.
XC0 = 0                                 # [1024, 512] = [256, 2048] x^T shard
WA0 = 1024                              # [C, 512]: qc | kc | vc | qr slices
WB0 = 2048                              # [512, 512]: kr (2 halves) | vr (2 halves)
MS0 = 2560                              # rows 0:4 score (4x512); rows 8:136
                                        # biases [128, 4]: cls m0,m1 | reg m0,m1
WL0 = 2696                              # [2C, 512]: wlin_cls | wlin_reg
BLOB_ROWS = 4744

RG = [list(range(N_CORES))]
B = ("cls", "reg")


def round_f32r(a: np.ndarray) -> np.ndarray:
    """Round-to-nearest-even at 11 explicit mantissa bits (= hardware f32r)."""
    u = np.ascontiguousarray(a, dtype=np.float32).view(np.uint32).astype(np.uint64)
    shift = np.uint64(12)
    bias = np.uint64((1 << 11) - 1)
    lsb = (u >> shift) & np.uint64(1)
    r = ((u + bias + lsb) >> shift) << shift
    return r.astype(np.uint32).view(np.float32).reshape(a.shape)


def build_nc(no_coll=False, phases=5):
    """Build the SPMD program (identical on every core; per-core data differs)."""
    nc = bacc.Bacc("TRN2", target_bir_lowering=False, debug=False,
                   num_devices=N_CORES)

    # ---- kernel I/O: single packed input blob + single packed output ----
    blob = nc.dram_tensor("blob", [BLOB_ROWS, 512], F16, kind="ExternalInput")
    out_t = nc.dram_tensor("out", [768, 512], F32, kind="ExternalOutput")
    bap = blob.ap()
    o_out = {"cls": out_t.ap()[0:256, :], "reg": out_t.ap()[256:512, :]}
    a_out = {"cls": out_t.ap()[512:640, :], "reg": out_t.ap()[640:768, :]}

    with tile.TileContext(nc) as tc:
        with tc.tile_pool(name="dram", bufs=1, space="DRAM") as dramp, \
             tc.tile_pool(name="const", bufs=1) as constp, \
             tc.tile_pool(name="persist", bufs=1) as persist:

            # ---- internal DRAM for collectives ----
            agx_in = dramp.tile([2 * 128, N2], F16, name="agx_in")
            agx_out = dramp.tile([2 * C, N2], F16, name="agx_out",
                                 addr_space="Shared")
            rs_in = dramp.tile([N_CORES * RS_BLK, N1], F16, name="rs_in")
            rs_out = dramp.tile([RS_BLK, N1], F16, name="rs_out")
            ag2a_in = dramp.tile([AG2A_ROWS, N1], F16, name="ag2a_in")
            ag2a_out = dramp.tile([N_CORES * AG2A_ROWS, N1], F16, name="ag2a_out",
                                  addr_space="Shared")
            ag2b_in = dramp.tile([AG2B_ROWS, N1], F16, name="ag2b_in")
            ag2b_out = dramp.tile([N_CORES * AG2B_ROWS, N1], F16, name="ag2b_out",
                                  addr_space="Shared")

            # gather the full x^T from per-core shards first
            nc.sync.dma_start(agx_in[:],
                              bap[XC0:XC0 + 1024, :]
                              .rearrange("(r f) n -> r (f n)", f=4))
            nc.gpsimd.collective_compute(
                "AllGather", mybir.AluOpType.bypass, replica_groups=RG,
                ins=[agx_in.opt()], outs=[agx_out.opt()])

            # ---- constants ----
            ones_f = constp.tile([128, 1], F32, name="ones_f")
            nc.vector.memset(ones_f[:], 1.0)
            ones = constp.tile([128, 1], F16, name="ones")
            nc.vector.tensor_copy(ones[:], ones_f[:])
            ones8 = constp.tile([8, 1], F16, name="ones8")
            nc.vector.tensor_copy(ones8[:], ones_f[0:8, :])
            eshift = constp.tile([128, 1], F32, name="eshift")
            nc.vector.memset(eshift[:], EXP_SHIFT)
            # (identity matrix no longer needed: transposes go via DMA xbar)
            score16 = constp.tile([1, N2], F16, name="score16")
            nc.sync.dma_start(score16[:].rearrange("o (f n) -> o f n", f=4),
                              bap[MS0:MS0 + 4, :])
            score_s = constp.tile([1, N2], F32, name="score_s")
            nc.vector.tensor_copy(score_s[:], score16[:])
            bias16 = constp.tile([128, 4], F16, name="bias16")
            nc.sync.dma_start(bias16[:], bap[MS0 + 8:MS0 + 136, 0:4])
            bias_s = {}
            for i, b in enumerate(B):
                bias_s[b] = constp.tile([128, 2], F32, name=f"bias_{b}",
                                        tag=f"bias_{b}")
                nc.vector.tensor_copy(bias_s[b][:], bias16[:, 2 * i:2 * i + 2])

            # ---- persistent SBUF (live until the end) ----
            vT512 = {b: persist.tile([128, N1], F16, name=f"vT512_{b}",
                                     tag=f"vT512_{b}") for b in B}
            vTok = {b: persist.tile([128, KT, 128], F16, name=f"vTok_{b}",
                                    tag=f"vTok_{b}") for b in B}

            # =========== Phases A+B under the k/v/q pool ===========
            with tc.tile_pool(name="ppool", bufs=1) as ppool:
                kS = {b: ppool.tile([128, KT, 128], F16, name=f"kS_{b}",
                                    tag=f"kS_{b}") for b in B}
                vN = {b: ppool.tile([128, KT, 128], F16, name=f"vN_{b}",
                                    tag=f"vN_{b}") for b in B}
                qN = {b: ppool.tile([128, N1], F16, name=f"qN_{b}",
                                    tag=f"qN_{b}") for b in B}

                # ---------------- Phase A: projections ----------------
                with tc.tile_pool(name="projw", bufs=1) as projw, \
                     tc.tile_pool(name="projx", bufs=2) as projx, \
                     tc.tile_pool(name="projtmp", bufs=2) as projtmp, \
                     tc.tile_pool(name="psA", bufs=3, space="PSUM") as psA, \
                     tc.tile_pool(name="psN", bufs=2, space="PSUM") as psN:

                    WA_SLOT = {("q", "cls"): 0, ("k", "cls"): 1,
                               ("v", "cls"): 2, ("q", "reg"): 3}
                    WB_SLOT = {("k", "reg"): 0, ("v", "reg"): 1}
                    for b in B:
                        w_s = {}
                        for t in ("q", "k", "v"):
                            w_s[t] = projw.tile([128, CC, HD], F16,
                                                name=f"w{t}", tag=f"w{t}")
                            if (t, b) in WA_SLOT:
                                j = WA_SLOT[t, b]
                                nc.sync.dma_start(
                                    w_s[t][:],
                                    bap[WA0:WA0 + C, j * 128:(j + 1) * 128]
                                    .rearrange("(c p) m -> p c m", p=128))
                            else:
                                j = WB_SLOT[t, b]
                                for hh in range(2):
                                    nc.sync.dma_start(
                                        w_s[t][:, 4 * hh:4 * hh + 4, :],
                                        bap[WB0:WB0 + 512,
                                            (2 * j + hh) * 128:
                                            (2 * j + hh + 1) * 128]
                                        .rearrange("(c p) m -> p c m", p=128))

                        for tt in range(TT):
                            xt_t = projx.tile([128, CC, 512], F16, name="xt",
                                              tag="xt")
                            ib = 0 if b == "cls" else 1
                            nc.sync.dma_start(
                                xt_t[:],
                                agx_out[:].rearrange("(c two p) n -> two p c n",
                                                     two=2, p=128)[ib]
                                [:, :, tt * 512:(tt + 1) * 512])

                            def proj(tname, xt_t=xt_t, w_s=w_s):
                                ps = psA.tile([128, 512], F32, name="proj",
                                              tag="proj")
                                for c in range(CC):
                                    nc.tensor.matmul(ps[:], w_s[tname][:, c, :],
                                                     xt_t[:, c, :],
                                                     start=(c == 0),
                                                     stop=(c == CC - 1))
                                return ps

                            def inv_norm(ps):
                                # 1/||col|| from a [128, 512] psum tile
                                sq = projtmp.tile([128, 512], F16, name="sq",
                                                  tag="sq")
                                nc.scalar.activation(sq[:], ps[:], AF.Square)
                                nsq = psN.tile([1, 512], F32, name="nsq", tag="nsq")
                                nc.tensor.matmul(nsq[:], ones[:], sq[:],
                                                 start=True, stop=True)
                                st = projtmp.tile([1, 512], F32, name="st", tag="st")
                                nc.scalar.activation(st[:], nsq[:], AF.Sqrt)
                                rt = projtmp.tile([1, 512], F32, name="rt", tag="rt")
                                nc.vector.reciprocal(rt[:], st[:])
                                return rt

                            def bcast(row):
                                bt = projtmp.tile([128, 512], F32, name="bc",
                                                  tag="bc")
                                nc.gpsimd.partition_broadcast(bt[:], row[:])
                                return bt

                            tsl = slice(tt * 4, (tt + 1) * 4)

                            # --- k: fold SCALE (and cls_score) and 1/|k| in ---
                            pk = proj("k")
                            rk = inv_norm(pk)
                            fk = projtmp.tile([1, 512], F32, name="fk", tag="fk")
                            nc.vector.tensor_scalar_mul(fk[:], rk[:], SCALE)
                            if b == "cls":
                                nc.vector.tensor_mul(
                                    fk[:], fk[:], score_s[:, tt * 512:(tt + 1) * 512])
                            nc.vector.tensor_mul(kS[b][:, tsl, :], pk[:], bcast(fk)[:])

                            # --- v: normalized copy + raw copy + transposes ---
                            pv = proj("v")
                            rv = inv_norm(pv)
                            nc.vector.tensor_mul(vN[b][:, tsl, :], pv[:], bcast(rv)[:])
                            vraw = (vT512[b] if tt == 0 else
                                    projtmp.tile([128, 512], F16, name="vraw",
                                                 tag="vraw"))
                            nc.scalar.activation(vraw[:], pv[:], AF.Copy)
                            for j in range(4):
                                nc.sync.dma_start_transpose(
                                    vTok[b][:, tt * 4 + j, :],
                                    vraw[:, j * 128:(j + 1) * 128])

                            # --- q (first token tile only) ---
                            if tt == 0:
                                pq = proj("q")
                                rq = inv_norm(pq)
                                nc.vector.tensor_mul(qN[b][:], pq[:], bcast(rq)[:])

                # ---------------- Phase B: attention + raw sims ----------------
                with tc.tile_pool(name="Ppool", bufs=1) as Ppool, \
                     tc.tile_pool(name="attnps", bufs=2, space="PSUM") as attnps, \
                     tc.tile_pool(name="accps", bufs=1, space="PSUM") as accps, \
                     tc.tile_pool(name="attntmp", bufs=2) as attntmp, \
                     tc.tile_pool(name="rhpool", bufs=1) as rhpool, \
                     tc.tile_pool(name="avgpool", bufs=3) as avgpool:
                    P = {b: Ppool.tile([128, KT, N1], F16, name=f"P_{b}",
                                       tag=f"P_{b}") for b in B}
                    xacc = {b: accps.tile([128, N1], F32, name=f"x_{b}",
                                          tag=f"x_{b}") for b in B}
                    dacc = {b: accps.tile([1, N1], F32, name=f"d_{b}",
                                          tag=f"d_{b}") for b in B}
                    for b in B:
                        for kt in range(KT):
                            s = attnps.tile([128, N1], F32, name="s", tag="s")
                            nc.tensor.matmul(s[:], kS[b][:, kt, :], qN[b][:],
                                             start=True, stop=True)
                            p_t = P[b][:, kt, :]
                            nc.scalar.activation(p_t, s[:], AF.Exp,
                                                 bias=eshift[:])
                            nc.tensor.matmul(dacc[b][:], ones[:], p_t,
                                             start=(kt == 0), stop=(kt == KT - 1))
                            # this head's raw v-v similarity for these keys;
                            # the RS head-sums it for the mask thresholds
                            rp = attnps.tile([128, N1], F32, name="rp", tag="rp")
                            nc.tensor.matmul(rp[:], vN[b][:, kt, :],
                                             vN[b][:, 0:4, :],
                                             start=True, stop=True)
                            rw = avgpool.tile([128, N1], F16, name="rw", tag="rw")
                            nc.scalar.activation(rw[:], rp[:], AF.Copy)
                            base = (kt // 2) * RS_BLK + (kt % 2) * 128
                            off = 256 if b == "cls" else 512
                            nc.sync.dma_start(
                                rs_in[base + off:base + off + 128, :], rw[:])

                    Rhalf = {}
                    for b in B:
                        d2 = attntmp.tile([1, N1], F32, name="d2", tag="d2")
                        nc.vector.tensor_scalar_mul(d2[:], dacc[b][:], 2.0)
                        rh = attntmp.tile([1, N1], F32, name="rh", tag="rh")
                        nc.vector.reciprocal(rh[:], d2[:])
                        Rhalf[b] = rhpool.tile([128, N1], F32, name=f"Rh_{b}",
                                               tag=f"Rh_{b}")
                        nc.gpsimd.partition_broadcast(Rhalf[b][:], rh[:])

                    # attn_avg^T = P_cls/(2 D_cls) + P_reg/(2 D_reg), bf16, to DRAM;
                    # x^T[b] = sum_kt vTok_b[kt] @ (P_cls'[kt] + P_reg'[kt])
                    for kt in range(KT):
                        for b in B:
                            nc.vector.tensor_mul(P[b][:, kt, :], P[b][:, kt, :],
                                                 Rhalf[b][:])
                        av = avgpool.tile([128, N1], F16, name="avg", tag="avg")
                        nc.vector.tensor_add(av[:], P["cls"][:, kt, :],
                                             P["reg"][:, kt, :])
                        base = (kt // 2) * RS_BLK + (kt % 2) * 128
                        nc.sync.dma_start(rs_in[base:base + 128, :], av[:])
                        for b in B:
                            for i2, b2 in enumerate(B):
                                nc.tensor.matmul(
                                    xacc[b][:], vTok[b][:, kt, :], P[b2][:, kt, :],
                                    start=(kt == 0 and i2 == 0),
                                    stop=(kt == KT - 1 and i2 == 1))
                    for b in B:
                        xs = attntmp.tile([128, N1], F16, name="xs", tag="xs")
                        nc.scalar.activation(xs[:], xacc[b][:], AF.Copy)
                        off = 0 if b == "cls" else 128
                        nc.sync.dma_start(ag2a_in[off:off + 128, :], xs[:])
                    for i, b in enumerate(B):
                        nc.sync.dma_start(
                            ag2a_in[256 + i * 128:256 + (i + 1) * 128, :],
                            vT512[b][:])

            nc.gpsimd.collective_compute(
                "ReduceScatter", mybir.AluOpType.add, replica_groups=RG,
                ins=[rs_in.opt()], outs=[rs_out.opt()])
            # x/v half of the exchange flies while the masks are computed
            nc.gpsimd.collective_compute(
                "AllGather", mybir.AluOpType.bypass, replica_groups=RG,
                ins=[ag2a_in.opt()], outs=[ag2a_out.opt()])

            # ======= Phase C/D: masks from reduced raw sims + masked exp =======
            with tc.tile_pool(name="vng", bufs=1) as vng, \
                 tc.tile_pool(name="dps", bufs=2, space="PSUM") as dps:
                asum = vng.tile([128, 2, N1], F16, name="asum")
                nc.sync.dma_start(
                    asum[:], rs_out[0:256, :].rearrange("(t p) q -> p t q", p=128))
                raw = {}
                for i, b in enumerate(B):
                    raw[b] = vng.tile([128, 2, N1], F16, name=f"raw_{b}",
                                      tag=f"raw_{b}")
                    nc.sync.dma_start(
                        raw[b][:],
                        rs_out[256 * (i + 1):256 * (i + 2), :]
                        .rearrange("(t p) q -> p t q", p=128))
                msk = {b: vng.tile([128, 2, N1], F16, name=f"msk_{b}",
                                   tag=f"msk_{b}") for b in B}
                for b, thr in (("cls", 0.75), ("reg", 0.99)):
                    for t in range(2):
                        nc.vector.tensor_scalar(
                            msk[b][:, t, :], raw[b][:, t, :], 1.0 / N_CORES, thr,
                            mybir.AluOpType.mult, mybir.AluOpType.is_gt)

                mes16 = vng.tile([128, 2, N1], F16, name="mes16")
                meo16 = vng.tile([128, 2, N1], F16, name="meo16")
                dp1 = dps.tile([1, N1], F32, name="dp1", tag="dp1")
                dp2 = dps.tile([1, N1], F32, name="dp2", tag="dp2")
                for t in range(2):
                    e_t = vng.tile([128, N1], F16, name=f"e_{t}", tag=f"e_{t}")
                    nc.scalar.activation(e_t[:], asum[:, t, :], AF.Exp,
                                         scale=1.0 / N_CORES)
                    nc.vector.tensor_mul(mes16[:, t, :], e_t[:],
                                         msk["cls"][:, t, :])
                    nc.vector.tensor_mul(meo16[:, t, :], mes16[:, t, :],
                                         msk["reg"][:, t, :])
                    nc.tensor.matmul(dp1[:], ones[:], mes16[:, t, :],
                                     start=(t == 0), stop=(t == 1))
                    nc.tensor.matmul(dp2[:], ones[:], meo16[:, t, :],
                                     start=(t == 0), stop=(t == 1))
                d1s = vng.tile([1, N1], F16, name="d1s")
                d2s = vng.tile([1, N1], F16, name="d2s")
                nc.scalar.activation(d1s[:], dp1[:], AF.Copy)
                nc.scalar.activation(d2s[:], dp2[:], AF.Copy)

                nc.sync.dma_start(
                    ag2b_in[0:512, :]
                    .rearrange("(x k p) q -> x p k q", x=2, p=128)[0],
                    mes16[:])
                nc.sync.dma_start(
                    ag2b_in[0:512, :]
                    .rearrange("(x k p) q -> x p k q", x=2, p=128)[1],
                    meo16[:])
                nc.sync.dma_start(ag2b_in[512:513, :], d1s[:])
                nc.sync.dma_start(ag2b_in[513:514, :], d2s[:])

            nc.gpsimd.collective_compute(
                "AllGather", mybir.AluOpType.bypass, replica_groups=RG,
                ins=[ag2b_in.opt()], outs=[ag2b_out.opt()])

            # ============ Phase E1: output linears ============
            with tc.tile_pool(name="lin", bufs=1) as lin, \
                 tc.tile_pool(name="linps", bufs=4, space="PSUM") as linps, \
                 tc.tile_pool(name="lintmp", bufs=2) as lintmp:
                XG = {b: lin.tile([128, N_CORES, N1], F16, name=f"XG_{b}",
                                  tag=f"XG_{b}") for b in B}
                VG = {b: lin.tile([128, N_CORES, N1], F16, name=f"VG_{b}",
                                  tag=f"VG_{b}") for b in B}
                for r in range(N_CORES):
                    base = r * AG2A_ROWS
                    for i, b in enumerate(B):
                        nc.sync.dma_start(
                            XG[b][:, r, :],
                            ag2a_out[base + i * 128:base + (i + 1) * 128, :])
                        nc.sync.dma_start(
                            VG[b][:, r, :],
                            ag2a_out[base + 256 + i * 128:
                                     base + 256 + (i + 1) * 128, :])

                wl_s = {}
                for b in B:
                    wl_s[b] = lin.tile([128, 2 * CC, 2, 128], F16, name=f"wl_{b}",
                                       tag=f"wl_{b}")  # plain W_lin col slice
                    i = 0 if b == "cls" else 1
                    nc.sync.dma_start(
                        wl_s[b][:],
                        bap[WL0:WL0 + 2 * C, i * 256:(i + 1) * 256]
                        .rearrange("(c p) (m u) -> p c m u", p=128, u=128))

                for b in B:
                    for m in range(2):
                        op_ = linps.tile([128, N1], F32, name="olin", tag="olin")
                        for c in range(2 * CC):
                            rhs = XG[b][:, c, :] if c < CC else VG[b][:, c - CC, :]
                            nc.tensor.matmul(op_[:], wl_s[b][:, c, m, :], rhs,
                                             start=(c == 0), stop=(c == 2 * CC - 1))
                        osb = lintmp.tile([128, N1], F32, name="osb", tag="osb")
                        nc.vector.tensor_scalar_add(osb[:], op_[:],
                                                    bias_s[b][:, m:m + 1])
                        nc.sync.dma_start(o_out[b][m * 128:(m + 1) * 128, :],
                                          osb[:])

            # ============ Phase E2: ave branch ============
            with tc.tile_pool(name="avp", bufs=1) as avp, \
                 tc.tile_pool(name="aveps", bufs=4, space="PSUM") as aveps, \
                 tc.tile_pool(name="avetmp", bufs=2) as avetmp:
                MS16 = {"cls": avp.tile([128, KT, N1], F16, name="MS16"),
                        "reg": avp.tile([128, KT, N1], F16, name="MO16")}
                DP16 = avp.tile([8, 2, N1], F16, name="DP16")
                for r in range(N_CORES):
                    base = r * AG2B_ROWS
                    nc.sync.dma_start(
                        MS16["cls"][:, 2 * r:2 * r + 2, :],
                        ag2b_out[base:base + 256, :]
                        .rearrange("(k p) q -> p k q", p=128))
                    nc.sync.dma_start(
                        MS16["reg"][:, 2 * r:2 * r + 2, :],
                        ag2b_out[base + 256:base + 512, :]
                        .rearrange("(k p) q -> p k q", p=128))
                    nc.sync.dma_start(
                        DP16[r:r + 1, :, :],
                        ag2b_out[base + 512:base + 514, :])

                Rd = {}
                for i, b in enumerate(B):
                    dsum = aveps.tile([1, N1], F32, name="dsum", tag="dsum")
                    nc.tensor.matmul(dsum[:], ones8[:], DP16[:, i, :],
                                     start=True, stop=True)
                    rr = avetmp.tile([1, N1], F32, name="rr", tag="rr")
                    nc.vector.reciprocal(rr[:], dsum[:])
                    Rd[b] = avetmp.tile([128, N1], F32, name=f"Rd_{b}",
                                        tag=f"Rd_{b}")
                    nc.gpsimd.partition_broadcast(Rd[b][:], rr[:])

                for b in B:
                    # columns of this head; support = own token-major v
                    ap_ = aveps.tile([128, N1], F32, name="avep", tag="avep")
                    for kt in range(KT):
                        nc.tensor.matmul(ap_[:], vTok[b][:, kt, :],
                                         MS16[b][:, kt, :],
                                         start=(kt == 0), stop=(kt == KT - 1))
                    asb = avetmp.tile([128, N1], F32, name="asb", tag="asb")
                    nc.vector.tensor_mul(asb[:], ap_[:], Rd[b][:])
                    nc.sync.dma_start(a_out[b], asb[:])

    nc.finalize()
    return nc


def make_in_maps(inputs: dict) -> list[dict]:
    """Host-side staging: pack per-core slices into one pre-rounded blob."""
    x_cls = np.asarray(inputs["x_cls"], np.float32)[0]      # [N2, C]
    x_reg = np.asarray(inputs["x_reg"], np.float32)[0]
    cls_score = np.asarray(inputs["cls_score"], np.float32)
    W_q = {"cls": np.asarray(inputs["W_q_cls"], np.float32),
           "reg": np.asarray(inputs["W_q_reg"], np.float32)}
    W_kv = {"cls": np.asarray(inputs["W_kv_cls"], np.float32),
            "reg": np.asarray(inputs["W_kv_reg"], np.float32)}
    W_l = {"cls": np.asarray(inputs["W_lin"], np.float32),
           "reg": np.asarray(inputs["W_lin_reg"], np.float32)}
    b_l = {"cls": np.asarray(inputs["b_lin"], np.float32),
           "reg": np.asarray(inputs["b_lin_reg"], np.float32)}

    xt = {b: np.ascontiguousarray(x.T).astype(np.float16)
          for b, x in (("cls", x_cls), ("reg", x_reg))}

    in_maps = []
    for h in range(N_CORES):
        hs = slice(h * HD, (h + 1) * HD)
        vs = slice(C + h * HD, C + (h + 1) * HD)
        blob = np.zeros((BLOB_ROWS, 512), np.float16)
        shard = np.concatenate([xt["cls"][h * HD:(h + 1) * HD],
                                xt["reg"][h * HD:(h + 1) * HD]], 0)
        blob[XC0:XC0 + 1024] = shard.reshape(1024, 512)
        wa = np.concatenate([W_q["cls"][:, hs], W_kv["cls"][:, hs],
                             W_kv["cls"][:, vs], W_q["reg"][:, hs]], 1)
        blob[WA0:WA0 + C] = wa.astype(np.float16)
        kr, vr = W_kv["reg"][:, hs], W_kv["reg"][:, vs]
        wb = np.concatenate([kr[0:512], kr[512:1024],
                             vr[0:512], vr[512:1024]], 1)
        blob[WB0:WB0 + 512] = wb.astype(np.float16)
        blob[MS0:MS0 + 4] = cls_score.reshape(4, 512).astype(np.float16)
        for i, b in enumerate(B):
            blob[MS0 + 8:MS0 + 136, 2 * i:2 * i + 2] = \
                b_l[b][h * 256:(h + 1) * 256].reshape(2, 128).T.astype(np.float16)
        wl = np.concatenate([W_l["cls"][:, h * 256:(h + 1) * 256],
                             W_l["reg"][:, h * 256:(h + 1) * 256]], 1)
        blob[WL0:WL0 + 2 * C] = wl.astype(np.float16)
        in_maps.append({"blob": blob})
    return in_maps


def assemble(results: list[dict]) -> tuple[np.ndarray, np.ndarray]:
    """Host-side gather of per-core column slices into the full features."""
    feats = []
    for i, b in enumerate(B):
        ave = np.concatenate(
            [results[c]["out"][512 + i * 128:512 + (i + 1) * 128].T
             for c in range(N_CORES)], 1)
        out = np.concatenate(
            [results[c]["out"][i * 256:(i + 1) * 256].T
             for c in range(N_CORES)], 1)
        feats.append(np.concatenate([ave, out], 1).astype(np.float32))
    return feats[0], feats[1]


_CACHE = {}


def get_nc():
    if "nc" not in _CACHE:
        _CACHE["nc"] = build_nc()
    return _CACHE["nc"]


class _Runner:
    """Cached jitted SPMD executor (mirrors bass2jax.run_bass_via_pjrt)."""

    def __init__(self, nc):
        import jax
        from jax.sharding import Mesh, PartitionSpec
        from jax.experimental.shard_map import shard_map
        from concourse.bass2jax import (_bass_exec_p, install_neuronx_cc_hook,
                                        partition_id_tensor)
        install_neuronx_cc_hook()
        self.jax = jax
        pname = nc.partition_id_tensor.name if nc.partition_id_tensor else None
        in_names, out_names, out_avals, zero_outs = [], [], [], []
        for alloc in nc.m.functions[0].allocations:
            if not isinstance(alloc, mybir.MemoryLocationSet):
                continue
            name = alloc.memorylocations[0].name
            if alloc.kind == "ExternalInput":
                if name != pname:
                    in_names.append(name)
            elif alloc.kind == "ExternalOutput":
                out_names.append(name)
                shape = tuple(alloc.tensor_shape)
                dtype = mybir.dt.np(alloc.dtype)
                out_avals.append(jax.core.ShapedArray(shape, dtype))
                zero_outs.append(np.zeros(shape, dtype))
        self.in_names, self.out_names = in_names, out_names
        self.out_avals, self.zero_outs = out_avals, zero_outs
        n_params, n_outs = len(in_names), len(out_names)
        all_in = in_names + out_names + ([pname] if pname else [])

        def _body(*args):
            operands = list(args)
            if pname is not None:
                operands.append(partition_id_tensor())
            return tuple(_bass_exec_p.bind(
                *operands, out_avals=tuple(out_avals), in_names=tuple(all_in),
                out_names=tuple(out_names), lowering_input_output_aliases=(),
                sim_require_finite=True, sim_require_nnan=True, nc=nc))

        devices = jax.devices()[:N_CORES]
        mesh = Mesh(np.asarray(devices), ("core",))
        self.fn = jax.jit(
            shard_map(_body, mesh=mesh,
                      in_specs=(PartitionSpec("core"),) * (n_params + n_outs),
                      out_specs=(PartitionSpec("core"),) * n_outs,
                      check_rep=False),
            keep_unused=True)

    def __call__(self, in_maps):
        n = N_CORES
        concat_in = [np.concatenate([np.asarray(in_maps[c][k]) for c in range(n)], 0)
                     for k in self.in_names]
        concat_zeros = [np.zeros((n * z.shape[0], *z.shape[1:]), z.dtype)
                        for z in self.zero_outs]
        outs = self.fn(*concat_in, *concat_zeros)
        self.jax.block_until_ready(outs)
        return [{name: np.asarray(outs[i]).reshape(n, *self.out_avals[i].shape)[c]
                 for i, name in enumerate(self.out_names)}
                for c in range(n)]


def get_runner():
    if "runner" not in _CACHE:
        _CACHE["runner"] = _Runner(get_nc())
    return _CACHE["runner"]


def kernel(**inputs) -> tuple[np.ndarray, np.ndarray]:
    results = get_runner()(make_in_maps(inputs))
    return assemble(results)

